# revision 28
# baseline (speedup 1.0000x reference)
"""ALiBi bias kernel for Trainium2, SPMD across 8 NeuronCores.

out[b, h, i, j] = scores[b, h, i, j] - slope[h] * (i - j)

(The `offset` input cancels: (i+off) - (j+off) == i - j exactly in f32 for
integer offsets well inside the f32 exact-integer range.)

Sharding: flatten [B, H] = [2, 16] -> 32 slices; each of the 8 cores owns 4
consecutive (b, h) slices (pure data/tensor parallel, no collectives). The
bias only depends on (h, i - j), so each core builds, on device, one bias
"strip" per local head: strip[p, c] = slope_h * (p - c + 1920), shape
[128, 3968] (gpsimd iota for the integer ramp, then a tensor_scalar_mul by
the per-core slopes input). For the row-tile starting at row r0, the bias
tile [128, 2048] is exactly strip[:, 1920-r0 : 1920-r0+2048], so the main
loop is DMA-in -> one DVE tensor_sub -> DMA-out per [128, 2048] tile; the
kernel is HBM-bandwidth-bound (~134 MB of DMA traffic per core).

Input DMAs issue on the Sync engine's HWDGE ring and output DMAs on the
Scalar engine's ring (the two physical HW-DGE rings): separating the read
and write streams measurably reduces both runtime and run-to-run variance
versus putting all DMAs on one ring.
"""

import numpy as np

_B, _H, _S = 2, 16, 2048
_NC = 8
_SPC = (_B * _H) // _NC  # slices (b,h pairs) per core = 4
_P = 128                 # SBUF partitions / row-tile height
_PAD = _S - _P           # 1920
_SW = _S + _PAD          # strip width 3968
_NRT = _S // _P          # row tiles per slice = 16

_CACHE = {}
_IMPL = "strips"  # "strips" | "stt"


def _build_nc(bufs=6, out_engine="scalar", grp=1, split_iota=False):
    if _IMPL == "stt":
        return _build_nc_stt(bufs)
    if _IMPL == "accum":
        return _build_nc_accum(bufs)
    import concourse.tile as tile
    from concourse import bacc, mybir

    f32 = mybir.dt.float32
    nc = bacc.Bacc("TRN2", target_bir_lowering=False, debug=False)
    scores_in = nc.declare_dram_parameter("scores", [_SPC, _S, _S], f32, isOutput=False)
    slopes_in = nc.declare_dram_parameter("slopes", [_P, _SPC], f32, isOutput=False)
    out_ext = nc.declare_dram_parameter("out", [_SPC, _S, _S], f32, isOutput=True)

    with tile.TileContext(nc) as tc:
        with (
            tc.tile_pool(name="setup", bufs=1) as sup,
            tc.tile_pool(name="strip", bufs=1) as sp,
            tc.tile_pool(name="inp", bufs=bufs) as ip,
            tc.tile_pool(name="outp", bufs=bufs) as op,
        ):
            # base[p, c] = p - c + PAD, exact small integers in f32
            base = sup.tile([_P, _SW], f32)
            # Generated in (optionally) two chunks, rightmost first: the first
            # row-tile's bias window is cols [PAD, SW), so producing that
            # region first unblocks the store stream earlier.
            chunks = [(_PAD, _SW - _PAD), (0, _PAD)] if split_iota else [(0, _SW)]
            for c0, w in chunks:
                nc.gpsimd.iota(
                    base[:, c0 : c0 + w],
                    pattern=[[-1, w]],
                    base=_PAD - c0,
                    channel_multiplier=1,
                    allow_small_or_imprecise_dtypes=True,
                )
            slopes = sup.tile([_P, _SPC], f32)
            nc.sync.dma_start(slopes[:], slopes_in[:])
            # strip slice for local head hl: slope_hl * base
            strips = sp.tile([_P, _SPC * _SW], f32)
            for c0, w in chunks:
                for hl in range(_SPC):
                    nc.vector.tensor_scalar_mul(
                        strips[:, hl * _SW + c0 : hl * _SW + c0 + w],
                        base[:, c0 : c0 + w],
                        slopes[:, hl : hl + 1],
                    )
            out_eng = nc.scalar if out_engine == "scalar" else nc.sync
            for hl in range(_SPC):
                for g in range(_NRT // grp):
                    r0 = g * grp * _P
                    t = ip.tile([_P, grp, _S], f32)
                    src_ap = scores_in[hl, r0 : r0 + grp * _P, :].rearrange(
                        "(t p) j -> p t j", p=_P
                    )
                    nc.sync.dma_start(t[:], src_ap)
                    o = op.tile([_P, grp, _S], f32)
                    for k in range(grp):
                        off = hl * _SW + (_PAD - (r0 + k * _P))
                        nc.vector.tensor_sub(
                            o[:, k, :], t[:, k, :], strips[:, off : off + _S]
                        )
                    dst_ap = out_ext[hl, r0 : r0 + grp * _P, :].rearrange(
                        "(t p) j -> p t j", p=_P
                    )
                    out_eng.dma_start(dst_ap, o[:])
    nc.compile()
    return nc


def _build_nc_accum(bufs=8):
    """DMA-side accumulate: tiles are pre-filled with the NEGATED bias window
    (DVE copy, off the critical path), then the scores DMA lands with
    accum_op=add (SDMA CCE), so each tile's store depends only on its load."""
    import concourse.tile as tile
    from concourse import bacc, mybir

    f32 = mybir.dt.float32
    nc = bacc.Bacc("TRN2", target_bir_lowering=False, debug=False)
    scores_in = nc.declare_dram_parameter("scores", [_SPC, _S, _S], f32, isOutput=False)
    slopes_in = nc.declare_dram_parameter("slopes", [_P, _SPC], f32, isOutput=False)
    out_ext = nc.declare_dram_parameter("out", [_SPC, _S, _S], f32, isOutput=True)

    with tile.TileContext(nc) as tc:
        with (
            tc.tile_pool(name="setup", bufs=1) as sup,
            tc.tile_pool(name="strip", bufs=1) as sp,
            tc.tile_pool(name="work", bufs=bufs) as wp,
        ):
            base = sup.tile([_P, _SW], f32)
            nc.gpsimd.iota(
                base[:],
                pattern=[[-1, _SW]],
                base=_PAD,
                channel_multiplier=1,
                allow_small_or_imprecise_dtypes=True,
            )
            slopes = sup.tile([_P, _SPC], f32)
            nc.sync.dma_start(slopes[:], slopes_in[:])
            # negated strip: (base * slope) * -1
            strips = sp.tile([_P, _SPC * _SW], f32)
            for hl in range(_SPC):
                nc.vector.tensor_scalar(
                    strips[:, hl * _SW : (hl + 1) * _SW],
                    base[:],
                    slopes[:, hl : hl + 1],
                    -1.0,
                    op0=mybir.AluOpType.mult,
                    op1=mybir.AluOpType.mult,
                )
            for hl in range(_SPC):
                for r in range(_NRT):
                    r0 = r * _P
                    off = hl * _SW + (_PAD - r0)
                    t = wp.tile([_P, _S], f32)
                    nc.vector.tensor_copy(t[:], strips[:, off : off + _S])
                    nc.gpsimd.dma_start(
                        t[:],
                        scores_in[hl, r0 : r0 + _P, :],
                        accum_op=mybir.AluOpType.add,
                    )
                    nc.scalar.dma_start(out_ext[hl, r0 : r0 + _P, :], t[:])
    nc.compile()
    return nc


def _build_nc_stt(bufs=4):
    """Fused variant: out = (scores - colv[p]) + jrow[j] in one DVE op per
    tile via scalar_tensor_tensor; no wide strip tensor needed."""
    import concourse.tile as tile
    from concourse import bacc, mybir

    f32 = mybir.dt.float32
    sub, add, mult = (
        mybir.AluOpType.subtract,
        mybir.AluOpType.add,
        mybir.AluOpType.mult,
    )
    nc = bacc.Bacc("TRN2", target_bir_lowering=False, debug=False)
    scores_in = nc.declare_dram_parameter("scores", [_SPC, _S, _S], f32, isOutput=False)
    slopes_in = nc.declare_dram_parameter("slopes", [_P, _SPC], f32, isOutput=False)
    out_ext = nc.declare_dram_parameter("out", [_SPC, _S, _S], f32, isOutput=True)

    with tile.TileContext(nc) as tc:
        with (
            tc.tile_pool(name="setup", bufs=1) as sup,
            tc.tile_pool(name="inp", bufs=bufs) as ip,
            tc.tile_pool(name="outp", bufs=bufs) as op,
        ):
            # iota_j[p, j] = j ; iota_rp[p, t] = 128*t + p
            iota_j = sup.tile([_P, _S], f32)
            nc.gpsimd.iota(
                iota_j[:],
                pattern=[[1, _S]],
                base=0,
                channel_multiplier=0,
                allow_small_or_imprecise_dtypes=True,
            )
            iota_rp = sup.tile([_P, _NRT], f32)
            nc.gpsimd.iota(
                iota_rp[:],
                pattern=[[_P, _NRT]],
                base=0,
                channel_multiplier=1,
                allow_small_or_imprecise_dtypes=True,
            )
            slopes = sup.tile([_P, _SPC], f32)
            nc.sync.dma_start(slopes[:], slopes_in[:])
            # jrow[p, hl*S + j] = slope_hl * j ; colv[p, hl*NRT + t] = slope_hl*(128t+p)
            jrows = sup.tile([_P, _SPC * _S], f32)
            colvs = sup.tile([_P, _SPC * _NRT], f32)
            for hl in range(_SPC):
                nc.vector.tensor_scalar_mul(
                    jrows[:, hl * _S : (hl + 1) * _S], iota_j[:], slopes[:, hl : hl + 1]
                )
                nc.vector.tensor_scalar_mul(
                    colvs[:, hl * _NRT : (hl + 1) * _NRT],
                    iota_rp[:],
                    slopes[:, hl : hl + 1],
                )
            for hl in range(_SPC):
                for r in range(_NRT):
                    r0 = r * _P
                    t = ip.tile([_P, _S], f32)
                    nc.sync.dma_start(t[:], scores_in[hl, r0 : r0 + _P, :])
                    o = op.tile([_P, _S], f32)
                    nc.vector.scalar_tensor_tensor(
                        o[:],
                        t[:],
                        colvs[:, hl * _NRT + r : hl * _NRT + r + 1],
                        jrows[:, hl * _S : (hl + 1) * _S],
                        op0=sub,
                        op1=add,
                    )
                    nc.sync.dma_start(out_ext[hl, r0 : r0 + _P, :], o[:])
    nc.compile()
    return nc


def _slopes_np():
    # slopes as the reference computes them (f32 throughout)
    slopes = (
        2.0 ** (-8.0 * np.arange(1, _H + 1, dtype=np.float32) / np.float32(_H))
    ).astype(np.float32)
    per_core = np.empty((_NC, _P, _SPC), dtype=np.float32)
    for core in range(_NC):
        for hl in range(_SPC):
            h = (core * _SPC + hl) % _H
            per_core[core, :, hl] = slopes[h]
    return per_core


def run(scores, offset=0, trace=False, trace_kwargs=None):
    """Run the SPMD kernel; returns (full_output, BassKernelResults)."""
    from concourse.bass_utils import run_bass_kernel_spmd

    scores = np.asarray(scores)
    assert scores.shape == (_B, _H, _S, _S) and scores.dtype == np.float32

    if "nc" not in _CACHE:
        _CACHE["nc"] = _build_nc()
        _CACHE["slopes"] = _slopes_np()
    nc = _CACHE["nc"]
    slopes = _CACHE["slopes"]

    flat = scores.reshape(_B * _H, _S, _S)
    in_maps = [
        {"scores": flat[c * _SPC : (c + 1) * _SPC], "slopes": slopes[c]}
        for c in range(_NC)
    ]
    res = run_bass_kernel_spmd(
        nc,
        in_maps,
        core_ids=list(range(_NC)),
        trace=trace,
        **(trace_kwargs or {}),
    )
    out = np.empty((_B * _H, _S, _S), dtype=np.float32)
    for c in range(_NC):
        out[c * _SPC : (c + 1) * _SPC] = res.results[c]["out"]
    return out.reshape(_B, _H, _S, _S), res


def kernel(scores, offset=0):
    try:
        out, _ = run(scores, offset=offset, trace=False)
    except Exception:
        # One retry: a transient NRT/device hiccup on the previous attempt
        # usually clears on a fresh execute.
        out, _ = run(scores, offset=offset, trace=False)
    return out


# revision 29
# speedup vs baseline: 1.0094x; 1.0094x over previous
"""ALiBi bias kernel for Trainium2, SPMD across 8 NeuronCores.

out[b, h, i, j] = scores[b, h, i, j] - slope[h] * (i - j)

(The `offset` input cancels: (i+off) - (j+off) == i - j exactly in f32 for
integer offsets well inside the f32 exact-integer range.)

Sharding: flatten [B, H] = [2, 16] -> 32 slices; each of the 8 cores owns 4
consecutive (b, h) slices (pure data/tensor parallel, no collectives). The
bias only depends on (h, i - j), so each core builds, on device, one bias
"strip" per local head: strip[p, c] = slope_h * (p - c + 1920), shape
[128, 3968] (gpsimd iota for the integer ramp, then a tensor_scalar_mul by
the per-core slopes input). For the row-tile starting at row r0, the bias
tile [128, 2048] is exactly strip[:, 1920-r0 : 1920-r0+2048], so the main
loop is DMA-in -> one DVE tensor_sub -> DMA-out per [128, 2048] tile; the
kernel is HBM-bandwidth-bound (~134 MB of DMA traffic per core).

Input DMAs issue on the Sync engine's HWDGE ring and output DMAs on the
Scalar engine's ring (the two physical HW-DGE rings): separating the read
and write streams measurably reduces both runtime and run-to-run variance
versus putting all DMAs on one ring.
"""

import numpy as np

_B, _H, _S = 2, 16, 2048
_NC = 8
_SPC = (_B * _H) // _NC  # slices (b,h pairs) per core = 4
_P = 128                 # SBUF partitions / row-tile height
_PAD = _S - _P           # 1920
_SW = _S + _PAD          # strip width 3968
_NRT = _S // _P          # row tiles per slice = 16

_CACHE = {}
_IMPL = "strips"  # "strips" | "stt"


def _build_nc(bufs=6, out_engine="scalar", grp=1, split_iota=False, ring_mode="split"):
    if _IMPL == "stt":
        return _build_nc_stt(bufs)
    if _IMPL == "accum":
        return _build_nc_accum(bufs)
    import concourse.tile as tile
    from concourse import bacc, mybir

    f32 = mybir.dt.float32
    nc = bacc.Bacc("TRN2", target_bir_lowering=False, debug=False)
    scores_in = nc.declare_dram_parameter("scores", [_SPC, _S, _S], f32, isOutput=False)
    slopes_in = nc.declare_dram_parameter("slopes", [_P, _SPC], f32, isOutput=False)
    out_ext = nc.declare_dram_parameter("out", [_SPC, _S, _S], f32, isOutput=True)

    with tile.TileContext(nc) as tc:
        with (
            tc.tile_pool(name="setup", bufs=1) as sup,
            tc.tile_pool(name="strip", bufs=1) as sp,
            tc.tile_pool(name="inp", bufs=bufs) as ip,
            tc.tile_pool(name="outp", bufs=bufs) as op,
        ):
            # base[p, c] = p - c + PAD, exact small integers in f32
            base = sup.tile([_P, _SW], f32)
            # Generated in (optionally) two chunks, rightmost first: the first
            # row-tile's bias window is cols [PAD, SW), so producing that
            # region first unblocks the store stream earlier.
            chunks = [(_PAD, _SW - _PAD), (0, _PAD)] if split_iota else [(0, _SW)]
            for c0, w in chunks:
                nc.gpsimd.iota(
                    base[:, c0 : c0 + w],
                    pattern=[[-1, w]],
                    base=_PAD - c0,
                    channel_multiplier=1,
                    allow_small_or_imprecise_dtypes=True,
                )
            slopes = sup.tile([_P, _SPC], f32)
            nc.sync.dma_start(slopes[:], slopes_in[:])
            # strip slice for local head hl: slope_hl * base
            strips = sp.tile([_P, _SPC * _SW], f32)
            for c0, w in chunks:
                for hl in range(_SPC):
                    nc.vector.tensor_scalar_mul(
                        strips[:, hl * _SW + c0 : hl * _SW + c0 + w],
                        base[:, c0 : c0 + w],
                        slopes[:, hl : hl + 1],
                    )
            out_eng = nc.scalar if out_engine == "scalar" else nc.sync
            idx = 0
            for hl in range(_SPC):
                for g in range(_NRT // grp):
                    r0 = g * grp * _P
                    t = ip.tile([_P, grp, _S], f32)
                    src_ap = scores_in[hl, r0 : r0 + grp * _P, :].rearrange(
                        "(t p) j -> p t j", p=_P
                    )
                    if ring_mode == "swap":
                        in_eng, o_eng = nc.scalar, nc.sync
                    elif ring_mode == "alt":
                        in_eng = nc.sync if idx % 2 == 0 else nc.scalar
                        o_eng = nc.scalar if idx % 2 == 0 else nc.sync
                    else:
                        in_eng, o_eng = nc.sync, out_eng
                    idx += 1
                    in_eng.dma_start(t[:], src_ap)
                    o = op.tile([_P, grp, _S], f32)
                    for k in range(grp):
                        off = hl * _SW + (_PAD - (r0 + k * _P))
                        nc.vector.tensor_sub(
                            o[:, k, :], t[:, k, :], strips[:, off : off + _S]
                        )
                    dst_ap = out_ext[hl, r0 : r0 + grp * _P, :].rearrange(
                        "(t p) j -> p t j", p=_P
                    )
                    o_eng.dma_start(dst_ap, o[:])
    nc.compile()
    return nc


def _build_nc_accum(bufs=8):
    """DMA-side accumulate: tiles are pre-filled with the NEGATED bias window
    (DVE copy, off the critical path), then the scores DMA lands with
    accum_op=add (SDMA CCE), so each tile's store depends only on its load."""
    import concourse.tile as tile
    from concourse import bacc, mybir

    f32 = mybir.dt.float32
    nc = bacc.Bacc("TRN2", target_bir_lowering=False, debug=False)
    scores_in = nc.declare_dram_parameter("scores", [_SPC, _S, _S], f32, isOutput=False)
    slopes_in = nc.declare_dram_parameter("slopes", [_P, _SPC], f32, isOutput=False)
    out_ext = nc.declare_dram_parameter("out", [_SPC, _S, _S], f32, isOutput=True)

    with tile.TileContext(nc) as tc:
        with (
            tc.tile_pool(name="setup", bufs=1) as sup,
            tc.tile_pool(name="strip", bufs=1) as sp,
            tc.tile_pool(name="work", bufs=bufs) as wp,
        ):
            base = sup.tile([_P, _SW], f32)
            nc.gpsimd.iota(
                base[:],
                pattern=[[-1, _SW]],
                base=_PAD,
                channel_multiplier=1,
                allow_small_or_imprecise_dtypes=True,
            )
            slopes = sup.tile([_P, _SPC], f32)
            nc.sync.dma_start(slopes[:], slopes_in[:])
            # negated strip: (base * slope) * -1
            strips = sp.tile([_P, _SPC * _SW], f32)
            for hl in range(_SPC):
                nc.vector.tensor_scalar(
                    strips[:, hl * _SW : (hl + 1) * _SW],
                    base[:],
                    slopes[:, hl : hl + 1],
                    -1.0,
                    op0=mybir.AluOpType.mult,
                    op1=mybir.AluOpType.mult,
                )
            for hl in range(_SPC):
                for r in range(_NRT):
                    r0 = r * _P
                    off = hl * _SW + (_PAD - r0)
                    t = wp.tile([_P, _S], f32)
                    nc.vector.tensor_copy(t[:], strips[:, off : off + _S])
                    nc.gpsimd.dma_start(
                        t[:],
                        scores_in[hl, r0 : r0 + _P, :],
                        accum_op=mybir.AluOpType.add,
                    )
                    nc.scalar.dma_start(out_ext[hl, r0 : r0 + _P, :], t[:])
    nc.compile()
    return nc


def _build_nc_stt(bufs=4):
    """Fused variant: out = (scores - colv[p]) + jrow[j] in one DVE op per
    tile via scalar_tensor_tensor; no wide strip tensor needed."""
    import concourse.tile as tile
    from concourse import bacc, mybir

    f32 = mybir.dt.float32
    sub, add, mult = (
        mybir.AluOpType.subtract,
        mybir.AluOpType.add,
        mybir.AluOpType.mult,
    )
    nc = bacc.Bacc("TRN2", target_bir_lowering=False, debug=False)
    scores_in = nc.declare_dram_parameter("scores", [_SPC, _S, _S], f32, isOutput=False)
    slopes_in = nc.declare_dram_parameter("slopes", [_P, _SPC], f32, isOutput=False)
    out_ext = nc.declare_dram_parameter("out", [_SPC, _S, _S], f32, isOutput=True)

    with tile.TileContext(nc) as tc:
        with (
            tc.tile_pool(name="setup", bufs=1) as sup,
            tc.tile_pool(name="inp", bufs=bufs) as ip,
            tc.tile_pool(name="outp", bufs=bufs) as op,
        ):
            # iota_j[p, j] = j ; iota_rp[p, t] = 128*t + p
            iota_j = sup.tile([_P, _S], f32)
            nc.gpsimd.iota(
                iota_j[:],
                pattern=[[1, _S]],
                base=0,
                channel_multiplier=0,
                allow_small_or_imprecise_dtypes=True,
            )
            iota_rp = sup.tile([_P, _NRT], f32)
            nc.gpsimd.iota(
                iota_rp[:],
                pattern=[[_P, _NRT]],
                base=0,
                channel_multiplier=1,
                allow_small_or_imprecise_dtypes=True,
            )
            slopes = sup.tile([_P, _SPC], f32)
            nc.sync.dma_start(slopes[:], slopes_in[:])
            # jrow[p, hl*S + j] = slope_hl * j ; colv[p, hl*NRT + t] = slope_hl*(128t+p)
            jrows = sup.tile([_P, _SPC * _S], f32)
            colvs = sup.tile([_P, _SPC * _NRT], f32)
            for hl in range(_SPC):
                nc.vector.tensor_scalar_mul(
                    jrows[:, hl * _S : (hl + 1) * _S], iota_j[:], slopes[:, hl : hl + 1]
                )
                nc.vector.tensor_scalar_mul(
                    colvs[:, hl * _NRT : (hl + 1) * _NRT],
                    iota_rp[:],
                    slopes[:, hl : hl + 1],
                )
            for hl in range(_SPC):
                for r in range(_NRT):
                    r0 = r * _P
                    t = ip.tile([_P, _S], f32)
                    nc.sync.dma_start(t[:], scores_in[hl, r0 : r0 + _P, :])
                    o = op.tile([_P, _S], f32)
                    nc.vector.scalar_tensor_tensor(
                        o[:],
                        t[:],
                        colvs[:, hl * _NRT + r : hl * _NRT + r + 1],
                        jrows[:, hl * _S : (hl + 1) * _S],
                        op0=sub,
                        op1=add,
                    )
                    nc.sync.dma_start(out_ext[hl, r0 : r0 + _P, :], o[:])
    nc.compile()
    return nc


def _slopes_np():
    # slopes as the reference computes them (f32 throughout)
    slopes = (
        2.0 ** (-8.0 * np.arange(1, _H + 1, dtype=np.float32) / np.float32(_H))
    ).astype(np.float32)
    per_core = np.empty((_NC, _P, _SPC), dtype=np.float32)
    for core in range(_NC):
        for hl in range(_SPC):
            h = (core * _SPC + hl) % _H
            per_core[core, :, hl] = slopes[h]
    return per_core


def run(scores, offset=0, trace=False, trace_kwargs=None):
    """Run the SPMD kernel; returns (full_output, BassKernelResults)."""
    from concourse.bass_utils import run_bass_kernel_spmd

    scores = np.asarray(scores)
    assert scores.shape == (_B, _H, _S, _S) and scores.dtype == np.float32

    if "nc" not in _CACHE:
        _CACHE["nc"] = _build_nc()
        _CACHE["slopes"] = _slopes_np()
    nc = _CACHE["nc"]
    slopes = _CACHE["slopes"]

    flat = scores.reshape(_B * _H, _S, _S)
    in_maps = [
        {"scores": flat[c * _SPC : (c + 1) * _SPC], "slopes": slopes[c]}
        for c in range(_NC)
    ]
    res = run_bass_kernel_spmd(
        nc,
        in_maps,
        core_ids=list(range(_NC)),
        trace=trace,
        **(trace_kwargs or {}),
    )
    out = np.empty((_B * _H, _S, _S), dtype=np.float32)
    for c in range(_NC):
        out[c * _SPC : (c + 1) * _SPC] = res.results[c]["out"]
    return out.reshape(_B, _H, _S, _S), res


def kernel(scores, offset=0):
    try:
        out, _ = run(scores, offset=offset, trace=False)
    except Exception:
        # One retry: a transient NRT/device hiccup on the previous attempt
        # usually clears on a fresh execute.
        out, _ = run(scores, offset=offset, trace=False)
    return out
